# revision 15
# baseline (speedup 1.0000x reference)
"""Trainium2 Bass kernel for the CCS primal-dual iteration (dense_cnn).

Algorithm (per image, 10 iterations):
    u = sigmoid(os)
    repeat 10x:
        a  = u[y+1,x] - u[y,x]            (forward diff y, zero-padded)
        b  = u[y,x+1] - u[y,x]            (forward diff x, zero-padded)
        q  = relu(q) - a*vf1 - b*vf0      (q carried pre-relu)
        p0 = relu(q)*vf1 ; p1 = relu(q)*vf0
        r  = os + (p0[y-1]-p0[y]) + (p1[x-1]-p1[x])
        u  = sigmoid(r)
    output = r

Sharding: pure data parallel, one image per NeuronCore (B=8 over 8 cores).

On-chip layout per core: image transposed to x-major; partition p holds
x = 128*b + p for block b in [0,8); free dim is (b, y) with y contiguous.
  - y-shifts: free-dim offset APs (guard columns hold zeros)
  - x-shifts: TensorE matmuls with banded/identity 128x128 matrices,
    accumulated in PSUM (block-crossing terms via block-shifted rhs APs)
"""

import sys

for _p in ("/opt/trn_rl_repo", "/root/.axon_site/_ro/trn_rl_repo"):
    if _p not in sys.path:
        sys.path.append(_p)

import numpy as np

import concourse.bass as bass
import concourse.bacc as bacc
import concourse.mybir as mybir
from concourse.tile import TileContext
from concourse import bass_utils

P = 128          # partitions
NB = 8           # x blocks per image (W=1024)
H = 1024         # image height (y)
YG = H + 2       # guarded y-run: [guard, y0..y1023, guard]
CH = 512         # chunk of y columns (one PSUM bank of fp32)
NCH = H // CH    # y-chunks per block
NITER = 10
F32 = mybir.dt.float32
N_CORES = 8

_CACHED = {}


def _build_lhs_pack() -> np.ndarray:
    """Constant 128x128 matrices, packed [128, 6*128] in order:
    ident, nident, fwd, bndf, bwd, bndb.
    matmul computes out[m, c] = sum_k lhsT[k, m] * rhs[k, c].
    """
    I = np.eye(P, dtype=np.float32)
    # forward x-diff within block: out[m] = rhs[m+1] - rhs[m]
    fwd = -I.copy()
    for m in range(P - 1):
        fwd[m + 1, m] = 1.0
    # forward boundary: out[127] += rhs_nextblock[0]
    bndf = np.zeros((P, P), dtype=np.float32)
    bndf[0, P - 1] = 1.0
    # backward x-diff: out[m] = rhs[m-1] - rhs[m]
    bwd = -I.copy()
    for m in range(1, P):
        bwd[m - 1, m] = 1.0
    # backward boundary: out[0] += rhs_prevblock[127]
    bndb = np.zeros((P, P), dtype=np.float32)
    bndb[P - 1, 0] = 1.0
    return np.concatenate([fwd, bndf, bwd, bndb], axis=1)


def _emit_kernel(nc: bass.Bass):
    sub = mybir.AluOpType.subtract
    mult = mybir.AluOpType.mult
    add = mybir.AluOpType.add
    sigmoid = mybir.ActivationFunctionType.Sigmoid

    relu = mybir.ActivationFunctionType.Relu

    os_d = nc.dram_tensor("osd", [P, NB * H], F32, kind="ExternalInput")
    vf0_d = nc.dram_tensor("vf0d", [P, NB * H], F32, kind="ExternalInput")
    vf1_d = nc.dram_tensor("vf1d", [P, NB * H], F32, kind="ExternalInput")
    lhs_d = nc.dram_tensor("lhsd", [P, 4 * P], F32, kind="ExternalInput")
    out_d = nc.dram_tensor("outd", [P, NB * H], F32, kind="ExternalOutput")

    with TileContext(nc) as tc:
        with (
            tc.tile_pool(name="big", bufs=1) as big,
            tc.tile_pool(name="tmpa", bufs=2) as tmpa,
            tc.tile_pool(name="tmpp", bufs=2) as tmpp,
            tc.tile_pool(name="ps", bufs=3, space="PSUM") as psp,
        ):
            os_sb = big.tile([P, NB * H], F32, tag="os")
            vf0_sb = big.tile([P, NB * H], F32, tag="vf0")
            vf1_sb = big.tile([P, NB * H], F32, tag="vf1")
            q_sb = big.tile([P, NB * H], F32, tag="q")
            u_sb = big.tile([P, NB * YG], F32, tag="u")
            p0_sb = big.tile([P, NB * YG], F32, tag="p0")
            lhs_sb = big.tile([P, 4 * P], F32, tag="lhs")

            fwd = lhs_sb[:, 0 * P:1 * P]
            bndf = lhs_sb[:, 1 * P:2 * P]
            bwd = lhs_sb[:, 2 * P:3 * P]
            bndb = lhs_sb[:, 3 * P:4 * P]

            # column helpers -------------------------------------------------
            def cc(b, yh):        # compact tensors (os/vf/q): chunk slice
                s = b * H + yh * CH
                return slice(s, s + CH)

            def gc(b, yh, off=0):  # guarded tensors (u/p0): chunk slice
                s = b * YG + 1 + yh * CH + off
                return slice(s, s + CH)

            def ccb(b):           # compact tensors: whole-block slice
                return slice(b * H, (b + 1) * H)

            def gcb(b, off=0):    # guarded tensors: whole-block slice
                s = b * YG + 1 + off
                return slice(s, s + H)

            nc.sync.dma_start(out=lhs_sb[:], in_=lhs_d[:])
            # initial zeros: q fully, u/p0 fully (covers guard columns)
            nc.vector.memset(q_sb[:], 0.0)
            nc.vector.memset(u_sb[:], 0.0)
            nc.gpsimd.memset(p0_sb[:], 0.0)

            # input loads, chunked so compute can start early
            for b in range(NB):
                for yh in range(NCH):
                    c = cc(b, yh)
                    nc.sync.dma_start(out=os_sb[:, c], in_=os_d[:, c])
            for b in range(NB):
                nc.sync.dma_start(out=vf1_sb[:, b * H:(b + 1) * H],
                                  in_=vf1_d[:, b * H:(b + 1) * H])
                nc.sync.dma_start(out=vf0_sb[:, b * H:(b + 1) * H],
                                  in_=vf0_d[:, b * H:(b + 1) * H])

            # u0 = sigmoid(os)
            for b in range(NB):
                for yh in range(NCH):
                    nc.scalar.activation(u_sb[:, gc(b, yh)], os_sb[:, cc(b, yh)],
                                         sigmoid)

            p1_tiles = {}
            for it in range(NITER):
                last = it == NITER - 1
                for b in range(NB):
                    cb = ccb(b)
                    gb = gcb(b)
                    u_b = u_sb[:, gb]
                    q_b = q_sb[:, cb]

                    # --- A: bx = Dx+ u  (PE, per psum chunk; weight-batched)
                    b_ps = [psp.tile([P, CH], F32, tag="bps", name=f"bps{yh}")
                            for yh in range(NCH)]
                    for yh in range(NCH):
                        nc.tensor.matmul(b_ps[yh][:], fwd, u_sb[:, gc(b, yh)],
                                         start=True, stop=(b == NB - 1))
                    if b < NB - 1:
                        for yh in range(NCH):
                            nc.tensor.matmul(b_ps[yh][:], bndf,
                                             u_sb[:, gc(b + 1, yh)],
                                             start=False, stop=True)

                    # --- B: a = Dy+ u  (DVE) ---
                    a_t = tmpa.tile([P, H], F32, tag="a")
                    nc.vector.tensor_tensor(out=a_t[:], in0=u_sb[:, gcb(b, 1)],
                                            in1=u_b, op=sub)
                    # --- C: a *= vf1  (GPSIMD) ---
                    nc.gpsimd.tensor_tensor(out=a_t[:], in0=a_t[:],
                                            in1=vf1_sb[:, cb], op=mult)
                    # --- E: a = q - a  (DVE; q holds relu'd state) ---
                    nc.vector.tensor_tensor(out=a_t[:], in0=q_b, in1=a_t[:],
                                            op=sub)
                    # --- D: q = bx * vf0  (DVE, psum src; q dead after E) ---
                    for yh in range(NCH):
                        nc.vector.tensor_tensor(out=q_sb[:, cc(b, yh)],
                                                in0=b_ps[yh][:],
                                                in1=vf0_sb[:, cc(b, yh)],
                                                op=mult)
                    # --- F: q = a - q  (GPSIMD) ---
                    nc.gpsimd.tensor_tensor(out=q_b, in0=a_t[:], in1=q_b, op=sub)
                    # --- R: q = relu(q)  (ACT, in place) ---
                    nc.scalar.activation(q_b, q_b, relu)
                    # --- G: p0 = q*vf1  (DVE) ---
                    nc.vector.tensor_tensor(out=p0_sb[:, gb], in0=q_b,
                                            in1=vf1_sb[:, cb], op=mult)
                    # --- E~ part 1: r = p1prev[x-1] boundary term (PE) ---
                    # Emitted before H so that with tmpp bufs=2 the slot
                    # H(b) reuses has already been read by bndb here.
                    r_ps = [psp.tile([P, CH], F32, tag="rps", name=f"rps{yh}")
                            for yh in range(NCH)]
                    if b > 0:
                        for yh in range(NCH):
                            nc.tensor.matmul(r_ps[yh][:], bndb,
                                             p1_tiles[(b - 1, yh)][:],
                                             start=True, stop=False)

                    # --- H: p1 = q*vf0  (GPSIMD, per psum chunk) ---
                    for yh in range(NCH):
                        p1_t = tmpp.tile([P, CH], F32, tag="p1")
                        nc.gpsimd.tensor_tensor(out=p1_t[:],
                                                in0=q_sb[:, cc(b, yh)],
                                                in1=vf0_sb[:, cc(b, yh)],
                                                op=mult)
                        p1_tiles[(b, yh)] = p1_t

                    # --- D~: a = p0[y-1] - p0[y]  (DVE, reuse a) ---
                    nc.vector.tensor_tensor(out=a_t[:], in0=p0_sb[:, gcb(b, -1)],
                                            in1=p0_sb[:, gb], op=sub)
                    # --- T: a += os  (GPSIMD) ---
                    nc.gpsimd.tensor_tensor(out=a_t[:], in0=a_t[:],
                                            in1=os_sb[:, cb], op=add)

                    # --- E~ part 2: r += p1[x-1] - p1[x] banded (PE) ---
                    for yh in range(NCH):
                        nc.tensor.matmul(r_ps[yh][:], bwd, p1_tiles[(b, yh)][:],
                                         start=(b == 0), stop=True)

                    # --- TT2 + J: r += (os + d~);  u = sigmoid(r) ---
                    for yh in range(NCH):
                        a_half = a_t[:, yh * CH:(yh + 1) * CH]
                        if not last:
                            nc.vector.tensor_tensor(out=r_ps[yh][:],
                                                    in0=r_ps[yh][:],
                                                    in1=a_half, op=add)
                            nc.scalar.activation(u_sb[:, gc(b, yh)],
                                                 r_ps[yh][:], sigmoid)
                        else:
                            # u is dead in the last iteration: stage r there
                            u_c = u_sb[:, gc(b, yh)]
                            nc.vector.tensor_tensor(out=u_c, in0=r_ps[yh][:],
                                                    in1=a_half, op=add)
                            nc.sync.dma_start(out=out_d[:, cc(b, yh)], in_=u_c)
    return nc


def _get_built():
    if "nc" not in _CACHED:
        nc = bacc.Bacc("TRN2")
        _emit_kernel(nc)
        nc.compile()
        _CACHED["nc"] = nc
        _CACHED["lhs"] = _build_lhs_pack()
    return _CACHED["nc"], _CACHED["lhs"]


def _to_core_layout(img: np.ndarray) -> np.ndarray:
    """[H(y), W(x)] f32 -> [P, NB*H] with [p, b*H+y] = img[y, 128b+p]."""
    t = np.ascontiguousarray(img.T)                 # [x, y]
    t = t.reshape(NB, P, H).transpose(1, 0, 2)      # [p, b, y]
    return np.ascontiguousarray(t.reshape(P, NB * H))


def _from_core_layout(flat: np.ndarray) -> np.ndarray:
    """[P, NB*H] -> [H, W]."""
    t = flat.reshape(P, NB, H).transpose(1, 0, 2)   # [b, p, y]
    return np.ascontiguousarray(t.reshape(NB * P, H).T)


def kernel(o: np.ndarray, vector_field: np.ndarray, _trace=False):
    assert o.shape == (8, 1, 1024, 1024) and vector_field.shape == (8, 1024, 2, 1024)
    nc, lhs = _get_built()
    in_maps = []
    for ci in range(N_CORES):
        osd = _to_core_layout(np.asarray(o[ci, 0], dtype=np.float32))
        vf0 = _to_core_layout(np.asarray(vector_field[ci, :, 0, :], dtype=np.float32))
        vf1 = _to_core_layout(np.asarray(vector_field[ci, :, 1, :], dtype=np.float32))
        in_maps.append({"osd": osd, "vf0d": vf0, "vf1d": vf1, "lhsd": lhs})

    res = bass_utils.run_bass_kernel_spmd(nc, in_maps, list(range(N_CORES)),
                                          trace=_trace)
    out = np.stack([_from_core_layout(res.results[ci]["outd"])
                    for ci in range(N_CORES)]).astype(np.float32)
    if _trace:
        return out, res
    return out


# revision 17
# speedup vs baseline: 1.1112x; 1.1112x over previous
"""Trainium2 Bass kernel for the CCS primal-dual iteration (dense_cnn).

Algorithm (per image, 10 iterations):
    u = sigmoid(os)
    repeat 10x:
        a  = u[y+1,x] - u[y,x]            (forward diff y, zero-padded)
        b  = u[y,x+1] - u[y,x]            (forward diff x, zero-padded)
        q  = relu(q) - a*vf1 - b*vf0      (q carried pre-relu)
        p0 = relu(q)*vf1 ; p1 = relu(q)*vf0
        r  = os + (p0[y-1]-p0[y]) + (p1[x-1]-p1[x])
        u  = sigmoid(r)
    output = r

Sharding: pure data parallel, one image per NeuronCore (B=8 over 8 cores).

On-chip layout per core: image transposed to x-major; partition p holds
x = 128*b + p for block b in [0,8); free dim is (b, y) with y contiguous.
  - y-shifts: free-dim offset APs (guard columns hold zeros)
  - x-shifts: TensorE matmuls with banded/identity 128x128 matrices,
    accumulated in PSUM (block-crossing terms via block-shifted rhs APs)
"""

import sys

for _p in ("/opt/trn_rl_repo", "/root/.axon_site/_ro/trn_rl_repo"):
    if _p not in sys.path:
        sys.path.append(_p)

import numpy as np

import concourse.bass as bass
import concourse.bacc as bacc
import concourse.mybir as mybir
from concourse.tile import TileContext
from concourse import bass_utils

P = 128          # partitions
NB = 8           # x blocks per image (W=1024)
H = 1024         # image height (y)
YG = H + 2       # guarded y-run: [guard, y0..y1023, guard]
CH = 512         # chunk of y columns (one PSUM bank of fp32)
NCH = H // CH    # y-chunks per block
NITER = 10
F32 = mybir.dt.float32
N_CORES = 8

_CACHED = {}


def _build_lhs_pack() -> np.ndarray:
    """Constant 128x128 matrices, packed [128, 6*128] in order:
    ident, nident, fwd, bndf, bwd, bndb.
    matmul computes out[m, c] = sum_k lhsT[k, m] * rhs[k, c].
    """
    I = np.eye(P, dtype=np.float32)
    nI = -I
    # forward x-diff within block: out[m] = rhs[m+1] - rhs[m]
    fwd = -I.copy()
    for m in range(P - 1):
        fwd[m + 1, m] = 1.0
    # forward boundary: out[127] += rhs_nextblock[0]
    bndf = np.zeros((P, P), dtype=np.float32)
    bndf[0, P - 1] = 1.0
    # backward x-diff: out[m] = rhs[m-1] - rhs[m]
    bwd = -I.copy()
    for m in range(1, P):
        bwd[m - 1, m] = 1.0
    # backward boundary: out[0] += rhs_prevblock[127]
    bndb = np.zeros((P, P), dtype=np.float32)
    bndb[P - 1, 0] = 1.0
    return np.concatenate([I, nI, fwd, bndf, bwd, bndb], axis=1)


def _emit_kernel(nc: bass.Bass):
    sub = mybir.AluOpType.subtract
    mult = mybir.AluOpType.mult
    add = mybir.AluOpType.add
    sigmoid = mybir.ActivationFunctionType.Sigmoid
    relu = mybir.ActivationFunctionType.Relu

    os_d = nc.dram_tensor("osd", [P, NB * H], F32, kind="ExternalInput")
    vf0_d = nc.dram_tensor("vf0d", [P, NB * H], F32, kind="ExternalInput")
    vf1_d = nc.dram_tensor("vf1d", [P, NB * H], F32, kind="ExternalInput")
    lhs_d = nc.dram_tensor("lhsd", [P, 6 * P], F32, kind="ExternalInput")
    out_d = nc.dram_tensor("outd", [P, NB * H], F32, kind="ExternalOutput")

    with TileContext(nc) as tc:
        with (
            tc.tile_pool(name="big", bufs=1) as big,
            tc.tile_pool(name="tmpa", bufs=2) as tmpa,
            tc.tile_pool(name="tmpp", bufs=3) as tmpp,
            tc.tile_pool(name="ps", bufs=3, space="PSUM") as psp,
        ):
            os_sb = big.tile([P, NB * H], F32, tag="os")
            vf0_sb = big.tile([P, NB * H], F32, tag="vf0")
            vf1_sb = big.tile([P, NB * H], F32, tag="vf1")
            q_sb = big.tile([P, NB * H], F32, tag="q")
            u_sb = big.tile([P, NB * YG], F32, tag="u")
            p0_sb = big.tile([P, NB * YG], F32, tag="p0")
            lhs_sb = big.tile([P, 6 * P], F32, tag="lhs")

            ident = lhs_sb[:, 0 * P:1 * P]
            nident = lhs_sb[:, 1 * P:2 * P]
            fwd = lhs_sb[:, 2 * P:3 * P]
            bndf = lhs_sb[:, 3 * P:4 * P]
            bwd = lhs_sb[:, 4 * P:5 * P]
            bndb = lhs_sb[:, 5 * P:6 * P]

            # column helpers -------------------------------------------------
            def cc(b, yh):        # compact tensors (os/vf/q): chunk slice
                s = b * H + yh * CH
                return slice(s, s + CH)

            def gc(b, yh, off=0):  # guarded tensors (u/p0): chunk slice
                s = b * YG + 1 + yh * CH + off
                return slice(s, s + CH)

            nc.sync.dma_start(out=lhs_sb[:], in_=lhs_d[:])
            # initial zeros: q fully, u/p0 fully (covers guard columns)
            nc.vector.memset(q_sb[:], 0.0)
            nc.vector.memset(u_sb[:], 0.0)
            nc.gpsimd.memset(p0_sb[:], 0.0)

            # input loads, chunked so compute can start early
            for b in range(NB):
                for yh in range(NCH):
                    c = cc(b, yh)
                    nc.sync.dma_start(out=os_sb[:, c], in_=os_d[:, c])
            for b in range(NB):
                nc.sync.dma_start(out=vf1_sb[:, b * H:(b + 1) * H],
                                  in_=vf1_d[:, b * H:(b + 1) * H])
                nc.sync.dma_start(out=vf0_sb[:, b * H:(b + 1) * H],
                                  in_=vf0_d[:, b * H:(b + 1) * H])

            # u0 = sigmoid(os)
            for b in range(NB):
                for yh in range(NCH):
                    nc.scalar.activation(u_sb[:, gc(b, yh)], os_sb[:, cc(b, yh)],
                                         sigmoid)

            p1_tiles = {}
            for it in range(NITER):
                last = it == NITER - 1
                for b in range(NB):
                    for yh in range(NCH):
                        c = cc(b, yh)
                        g = gc(b, yh)
                        u_c = u_sb[:, g]
                        vf0_c = vf0_sb[:, c]
                        vf1_c = vf1_sb[:, c]
                        q_c = q_sb[:, c]

                        # --- A: b = Dx+ u  (PE) ---
                        b_ps = psp.tile([P, CH], F32, tag="bps")
                        nc.tensor.matmul(b_ps[:], fwd, u_c,
                                         start=True, stop=(b == NB - 1))
                        if b < NB - 1:
                            nc.tensor.matmul(b_ps[:], bndf,
                                             u_sb[:, gc(b + 1, yh)],
                                             start=False, stop=True)

                        # --- B: a = Dy+ u  (DVE) ---
                        a_t = tmpa.tile([P, CH], F32, tag="a")
                        nc.vector.tensor_tensor(out=a_t[:], in0=u_sb[:, gc(b, yh, 1)],
                                                in1=u_c, op=sub)
                        # --- C: a *= vf1  (GPSIMD) ---
                        nc.gpsimd.tensor_tensor(out=a_t[:], in0=a_t[:], in1=vf1_c,
                                                op=mult)
                        # --- E: a = q - a  (DVE; q holds relu'd state) ---
                        nc.vector.tensor_tensor(out=a_t[:], in0=q_c, in1=a_t[:],
                                                op=sub)
                        # --- D: q = b_ps * vf0  (DVE, psum src; q is dead
                        #        once E has read it) ---
                        nc.vector.tensor_tensor(out=q_c, in0=b_ps[:], in1=vf0_c,
                                                op=mult)
                        # --- F: q = a - q  (GPSIMD) ---
                        nc.gpsimd.tensor_tensor(out=q_c, in0=a_t[:], in1=q_c,
                                                op=sub)
                        # --- R: q = relu(q)  (ACT, in place) ---
                        nc.scalar.activation(q_c, q_c, relu)
                        # --- G: p0 = q*vf1  (DVE) ---
                        nc.vector.tensor_tensor(out=p0_sb[:, g], in0=q_c,
                                                in1=vf1_c, op=mult)
                        # --- H: p1 = q*vf0  (DVE) ---
                        p1_t = tmpp.tile([P, CH], F32, tag="p1")
                        nc.vector.tensor_tensor(out=p1_t[:], in0=q_c,
                                                in1=vf0_c, op=mult)
                        p1_tiles[(b, yh)] = p1_t

                        # --- I: r = (p0[y-1]-p0[y]) + (p1[x-1]-p1[x])  (PE)
                        r_ps = psp.tile([P, CH], F32, tag="rps")
                        nc.tensor.matmul(r_ps[:], ident, p0_sb[:, gc(b, yh, -1)],
                                         start=True, stop=False)
                        nc.tensor.matmul(r_ps[:], nident, p0_sb[:, g],
                                         start=False, stop=False)
                        nc.tensor.matmul(r_ps[:], bwd, p1_t[:],
                                         start=False, stop=(b == 0))
                        if b > 0:
                            nc.tensor.matmul(r_ps[:], bndb,
                                             p1_tiles[(b - 1, yh)][:],
                                             start=False, stop=True)

                        # --- TT2 + J: r += os;  u = sigmoid(r) ---
                        if not last:
                            nc.vector.tensor_tensor(out=r_ps[:], in0=r_ps[:],
                                                    in1=os_sb[:, c], op=add)
                            nc.scalar.activation(u_c, r_ps[:], sigmoid)
                        else:
                            # u is dead in the last iteration: stage r there
                            nc.vector.tensor_tensor(out=u_c, in0=r_ps[:],
                                                    in1=os_sb[:, c], op=add)
                            nc.sync.dma_start(out=out_d[:, c], in_=u_c)
    return nc


def _get_built():
    if "nc" not in _CACHED:
        nc = bacc.Bacc("TRN2")
        _emit_kernel(nc)
        nc.compile()
        _CACHED["nc"] = nc
        _CACHED["lhs"] = _build_lhs_pack()
    return _CACHED["nc"], _CACHED["lhs"]


def _to_core_layout(img: np.ndarray) -> np.ndarray:
    """[H(y), W(x)] f32 -> [P, NB*H] with [p, b*H+y] = img[y, 128b+p]."""
    t = np.ascontiguousarray(img.T)                 # [x, y]
    t = t.reshape(NB, P, H).transpose(1, 0, 2)      # [p, b, y]
    return np.ascontiguousarray(t.reshape(P, NB * H))


def _from_core_layout(flat: np.ndarray) -> np.ndarray:
    """[P, NB*H] -> [H, W]."""
    t = flat.reshape(P, NB, H).transpose(1, 0, 2)   # [b, p, y]
    return np.ascontiguousarray(t.reshape(NB * P, H).T)


def kernel(o: np.ndarray, vector_field: np.ndarray, _trace=False):
    assert o.shape == (8, 1, 1024, 1024) and vector_field.shape == (8, 1024, 2, 1024)
    nc, lhs = _get_built()
    in_maps = []
    for ci in range(N_CORES):
        osd = _to_core_layout(np.asarray(o[ci, 0], dtype=np.float32))
        vf0 = _to_core_layout(np.asarray(vector_field[ci, :, 0, :], dtype=np.float32))
        vf1 = _to_core_layout(np.asarray(vector_field[ci, :, 1, :], dtype=np.float32))
        in_maps.append({"osd": osd, "vf0d": vf0, "vf1d": vf1, "lhsd": lhs})

    res = bass_utils.run_bass_kernel_spmd(nc, in_maps, list(range(N_CORES)),
                                          trace=_trace)
    out = np.stack([_from_core_layout(res.results[ci]["outd"])
                    for ci in range(N_CORES)]).astype(np.float32)
    if _trace:
        return out, res
    return out


# revision 18
# speedup vs baseline: 1.1508x; 1.0356x over previous
"""Trainium2 Bass kernel for the CCS primal-dual iteration (dense_cnn).

Algorithm (per image, 10 iterations):
    u = sigmoid(os)
    repeat 10x:
        a  = u[y+1,x] - u[y,x]            (forward diff y, zero-padded)
        b  = u[y,x+1] - u[y,x]            (forward diff x, zero-padded)
        q  = relu(q) - a*vf1 - b*vf0      (q carried pre-relu)
        p0 = relu(q)*vf1 ; p1 = relu(q)*vf0
        r  = os + (p0[y-1]-p0[y]) + (p1[x-1]-p1[x])
        u  = sigmoid(r)
    output = r

Sharding: pure data parallel, one image per NeuronCore (B=8 over 8 cores).

On-chip layout per core: image transposed to x-major; partition p holds
x = 128*b + p for block b in [0,8); free dim is (b, y) with y contiguous.
  - y-shifts: free-dim offset APs (guard columns hold zeros)
  - x-shifts: TensorE matmuls with banded/identity 128x128 matrices,
    accumulated in PSUM (block-crossing terms via block-shifted rhs APs)
"""

import sys

for _p in ("/opt/trn_rl_repo", "/root/.axon_site/_ro/trn_rl_repo"):
    if _p not in sys.path:
        sys.path.append(_p)

import numpy as np

import concourse.bass as bass
import concourse.bacc as bacc
import concourse.mybir as mybir
from concourse.tile import TileContext
from concourse import bass_utils

P = 128          # partitions
NB = 8           # x blocks per image (W=1024)
H = 1024         # image height (y)
YG = H + 2       # guarded y-run: [guard, y0..y1023, guard]
CH = 512         # chunk of y columns (one PSUM bank of fp32)
NCH = H // CH    # y-chunks per block
NITER = 10
F32 = mybir.dt.float32
N_CORES = 8

_CACHED = {}


def _build_lhs_pack() -> np.ndarray:
    """Constant 128x128 matrices, packed [128, 6*128] in order:
    ident, nident, fwd, bndf, bwd, bndb.
    matmul computes out[m, c] = sum_k lhsT[k, m] * rhs[k, c].
    """
    I = np.eye(P, dtype=np.float32)
    nI = -I
    # forward x-diff within block: out[m] = rhs[m+1] - rhs[m]
    fwd = -I.copy()
    for m in range(P - 1):
        fwd[m + 1, m] = 1.0
    # forward boundary: out[127] += rhs_nextblock[0]
    bndf = np.zeros((P, P), dtype=np.float32)
    bndf[0, P - 1] = 1.0
    # backward x-diff: out[m] = rhs[m-1] - rhs[m]
    bwd = -I.copy()
    for m in range(1, P):
        bwd[m - 1, m] = 1.0
    # backward boundary: out[0] += rhs_prevblock[127]
    bndb = np.zeros((P, P), dtype=np.float32)
    bndb[P - 1, 0] = 1.0
    return np.concatenate([I, nI, fwd, bndf, bwd, bndb], axis=1)


def _emit_kernel(nc: bass.Bass):
    sub = mybir.AluOpType.subtract
    mult = mybir.AluOpType.mult
    amax = mybir.AluOpType.max
    sigmoid = mybir.ActivationFunctionType.Sigmoid

    os_d = nc.dram_tensor("osd", [P, NB * H], F32, kind="ExternalInput")
    vf0_d = nc.dram_tensor("vf0d", [P, NB * H], F32, kind="ExternalInput")
    vf1_d = nc.dram_tensor("vf1d", [P, NB * H], F32, kind="ExternalInput")
    lhs_d = nc.dram_tensor("lhsd", [P, 6 * P], F32, kind="ExternalInput")
    out_d = nc.dram_tensor("outd", [P, NB * H], F32, kind="ExternalOutput")

    with TileContext(nc) as tc:
        with (
            tc.tile_pool(name="big", bufs=1) as big,
            tc.tile_pool(name="tmpa", bufs=2) as tmpa,
            tc.tile_pool(name="tmpp", bufs=3) as tmpp,
            tc.tile_pool(name="ps", bufs=3, space="PSUM") as psp,
        ):
            os_sb = big.tile([P, NB * H], F32, tag="os")
            vf0_sb = big.tile([P, NB * H], F32, tag="vf0")
            vf1_sb = big.tile([P, NB * H], F32, tag="vf1")
            q_sb = big.tile([P, NB * H], F32, tag="q")
            u_sb = big.tile([P, NB * YG], F32, tag="u")
            p0_sb = big.tile([P, NB * YG], F32, tag="p0")
            lhs_sb = big.tile([P, 6 * P], F32, tag="lhs")

            ident = lhs_sb[:, 0 * P:1 * P]
            nident = lhs_sb[:, 1 * P:2 * P]
            fwd = lhs_sb[:, 2 * P:3 * P]
            bndf = lhs_sb[:, 3 * P:4 * P]
            bwd = lhs_sb[:, 4 * P:5 * P]
            bndb = lhs_sb[:, 5 * P:6 * P]

            # column helpers -------------------------------------------------
            def cc(b, yh):        # compact tensors (os/vf/q): chunk slice
                s = b * H + yh * CH
                return slice(s, s + CH)

            def gc(b, yh, off=0):  # guarded tensors (u/p0): chunk slice
                s = b * YG + 1 + yh * CH + off
                return slice(s, s + CH)

            nc.sync.dma_start(out=lhs_sb[:], in_=lhs_d[:])
            # initial zeros: q fully, u/p0 fully (covers guard columns)
            nc.vector.memset(q_sb[:], 0.0)
            nc.vector.memset(u_sb[:], 0.0)
            nc.gpsimd.memset(p0_sb[:], 0.0)

            # input loads, chunked so compute can start early
            for b in range(NB):
                for yh in range(NCH):
                    c = cc(b, yh)
                    nc.sync.dma_start(out=os_sb[:, c], in_=os_d[:, c])
            for b in range(NB):
                nc.sync.dma_start(out=vf1_sb[:, b * H:(b + 1) * H],
                                  in_=vf1_d[:, b * H:(b + 1) * H])
                nc.sync.dma_start(out=vf0_sb[:, b * H:(b + 1) * H],
                                  in_=vf0_d[:, b * H:(b + 1) * H])

            # u0 = sigmoid(os)
            for b in range(NB):
                for yh in range(NCH):
                    nc.scalar.activation(u_sb[:, gc(b, yh)], os_sb[:, cc(b, yh)],
                                         sigmoid)

            p1_tiles = {}
            for it in range(NITER):
                last = it == NITER - 1
                for b in range(NB):
                    for yh in range(NCH):
                        c = cc(b, yh)
                        g = gc(b, yh)
                        u_c = u_sb[:, g]
                        vf0_c = vf0_sb[:, c]
                        vf1_c = vf1_sb[:, c]
                        q_c = q_sb[:, c]

                        # --- A: b = Dx+ u  (PE) ---
                        b_ps = psp.tile([P, CH], F32, tag="bps")
                        nc.tensor.matmul(b_ps[:], fwd, u_c,
                                         start=True, stop=(b == NB - 1))
                        if b < NB - 1:
                            nc.tensor.matmul(b_ps[:], bndf,
                                             u_sb[:, gc(b + 1, yh)],
                                             start=False, stop=True)

                        # --- B: a = Dy+ u  (DVE) ---
                        a_t = tmpa.tile([P, CH], F32, tag="a")
                        nc.vector.tensor_tensor(out=a_t[:], in0=u_sb[:, gc(b, yh, 1)],
                                                in1=u_c, op=sub)
                        # --- C: a *= vf1  (GPSIMD) ---
                        nc.gpsimd.tensor_tensor(out=a_t[:], in0=a_t[:], in1=vf1_c,
                                                op=mult)
                        # --- E: a = relu(q) - a  (DVE stt) ---
                        nc.vector.scalar_tensor_tensor(out=a_t[:], in0=q_c,
                                                       scalar=0.0, in1=a_t[:],
                                                       op0=amax, op1=sub)
                        # --- D: q = b_ps * vf0  (DVE, psum src; q is dead
                        #        once E has read it) ---
                        nc.vector.tensor_tensor(out=q_c, in0=b_ps[:], in1=vf0_c,
                                                op=mult)
                        # --- F: q = a - q  (GPSIMD) ---
                        nc.gpsimd.tensor_tensor(out=q_c, in0=a_t[:], in1=q_c,
                                                op=sub)
                        # --- G: p0 = relu(q)*vf1  (DVE stt) ---
                        nc.vector.scalar_tensor_tensor(out=p0_sb[:, g], in0=q_c,
                                                       scalar=0.0, in1=vf1_c,
                                                       op0=amax, op1=mult)
                        # --- H: p1 = relu(q)*vf0  (DVE stt) ---
                        p1_t = tmpp.tile([P, CH], F32, tag="p1")
                        nc.vector.scalar_tensor_tensor(out=p1_t[:], in0=q_c,
                                                       scalar=0.0, in1=vf0_c,
                                                       op0=amax, op1=mult)
                        p1_tiles[(b, yh)] = p1_t

                        # --- I: r = os + (p0[y-1]-p0[y]) + (p1[x-1]-p1[x]) (PE)
                        r_ps = psp.tile([P, CH], F32, tag="rps")
                        nc.tensor.matmul(r_ps[:], ident, os_sb[:, c],
                                         start=True, stop=False)
                        nc.tensor.matmul(r_ps[:], ident, p0_sb[:, gc(b, yh, -1)],
                                         start=False, stop=False)
                        nc.tensor.matmul(r_ps[:], nident, p0_sb[:, g],
                                         start=False, stop=False)
                        nc.tensor.matmul(r_ps[:], bwd, p1_t[:],
                                         start=False, stop=(b == 0))
                        if b > 0:
                            nc.tensor.matmul(r_ps[:], bndb,
                                             p1_tiles[(b - 1, yh)][:],
                                             start=False, stop=True)

                        # --- J: u = sigmoid(r) / final store ---
                        if not last:
                            nc.scalar.activation(u_c, r_ps[:], sigmoid)
                        else:
                            # u is dead in the last iteration: stage r there
                            nc.scalar.copy(u_c, r_ps[:])
                            nc.sync.dma_start(out=out_d[:, c], in_=u_c)
    return nc


def _get_built():
    if "nc" not in _CACHED:
        nc = bacc.Bacc("TRN2")
        _emit_kernel(nc)
        nc.compile()
        _CACHED["nc"] = nc
        _CACHED["lhs"] = _build_lhs_pack()
    return _CACHED["nc"], _CACHED["lhs"]


def _to_core_layout(img: np.ndarray) -> np.ndarray:
    """[H(y), W(x)] f32 -> [P, NB*H] with [p, b*H+y] = img[y, 128b+p]."""
    t = np.ascontiguousarray(img.T)                 # [x, y]
    t = t.reshape(NB, P, H).transpose(1, 0, 2)      # [p, b, y]
    return np.ascontiguousarray(t.reshape(P, NB * H))


def _from_core_layout(flat: np.ndarray) -> np.ndarray:
    """[P, NB*H] -> [H, W]."""
    t = flat.reshape(P, NB, H).transpose(1, 0, 2)   # [b, p, y]
    return np.ascontiguousarray(t.reshape(NB * P, H).T)


def kernel(o: np.ndarray, vector_field: np.ndarray, _trace=False):
    assert o.shape == (8, 1, 1024, 1024) and vector_field.shape == (8, 1024, 2, 1024)
    nc, lhs = _get_built()
    in_maps = []
    for ci in range(N_CORES):
        osd = _to_core_layout(np.asarray(o[ci, 0], dtype=np.float32))
        vf0 = _to_core_layout(np.asarray(vector_field[ci, :, 0, :], dtype=np.float32))
        vf1 = _to_core_layout(np.asarray(vector_field[ci, :, 1, :], dtype=np.float32))
        in_maps.append({"osd": osd, "vf0d": vf0, "vf1d": vf1, "lhsd": lhs})

    res = bass_utils.run_bass_kernel_spmd(nc, in_maps, list(range(N_CORES)),
                                          trace=_trace)
    out = np.stack([_from_core_layout(res.results[ci]["outd"])
                    for ci in range(N_CORES)]).astype(np.float32)
    if _trace:
        return out, res
    return out
